# revision 13
# baseline (speedup 1.0000x reference)
"""CSRN (spatial GRU scan network) Trainium2 kernel.

Sharding: data-parallel over batch across 8 cores (4 batches/core).
Each core runs both directional scans (row + col) fused: 8 chains
(2 scans x 4 batches) stepped together, 512 sequential steps.

Host-side prep (not counted in HW time): packs x into a bf16 tensor
holding both the natural (row-scan) and transposed (col-scan) layouts
as matmul-ready contraction rows plus a constant ones-row (for biases),
and packs the tiny GRU/conv/combine weights into block-structured lhsT
matrices.

Layout rules honored: engine SBUF access patterns start at partition
0/32/64/96 only (PSUM operands are exempt); matmul lhsT/rhs share a
32-aligned base partition; one matmul output <= one PSUM bank (N<=512).
"""

import sys

sys.path.insert(0, "/opt/trn_rl_repo")

import numpy as np
import ml_dtypes

BF16 = ml_dtypes.bfloat16

C = 3  # channels
B = 4  # batches per core
NCORES = 8
CH = 2 * B  # chains per core (2 scans x 4 batches)
RW = CH * C  # 24 data rows per class
KG = 56  # gate-matmul contraction rows: x 0-23, ones 24, (pad), h 32-55
MG = 128  # gate M rows: r 0-23, z 32-55, inn 64-87, hnh 96-119 (rest pad)
NSLOT = 32  # xh ring slots
CHUNK = 16  # x-stream DMA chunk (steps)


def _pack_weights(inp):
    """Build lhsT matrices (numpy fp32 -> bf16). Row order everywhere:
    chain = scan*4 + b (scan 0 = row scan, 1 = col scan), row = 3*chain + c."""
    w_ih = [inp["w_ih_rows"], inp["w_ih_cols"]]
    w_hh = [inp["w_hh_rows"], inp["w_hh_cols"]]
    b_ih = [inp["b_ih_rows"], inp["b_ih_cols"]]
    b_hh = [inp["b_hh_rows"], inp["b_hh_cols"]]
    cv_w = [inp["conv_rows_w"], inp["conv_cols_w"]]
    cv_b = [inp["conv_rows_b"], inp["conv_cols_b"]]
    cb_w = inp["combine_w"]  # (C, 2C)
    cb_b = inp["combine_b"]  # (C,)

    wg = np.zeros((KG, MG), np.float32)
    mbase = {"r": 0, "z": 32, "inn": 64, "hnh": 96}
    for ch in range(CH):
        sc = ch // B
        for ci in range(C):
            kx = 3 * ch + ci  # x row
            kh = 32 + 3 * ch + ci  # h row
            for co in range(C):
                wg[kx, mbase["r"] + 3 * ch + co] = w_ih[sc][0 + co, ci]
                wg[kh, mbase["r"] + 3 * ch + co] = w_hh[sc][0 + co, ci]
                wg[kx, mbase["z"] + 3 * ch + co] = w_ih[sc][3 + co, ci]
                wg[kh, mbase["z"] + 3 * ch + co] = w_hh[sc][3 + co, ci]
                wg[kx, mbase["inn"] + 3 * ch + co] = w_ih[sc][6 + co, ci]
                wg[kh, mbase["hnh"] + 3 * ch + co] = w_hh[sc][6 + co, ci]
        for co in range(C):  # biases via the ones row (k=24)
            wg[24, mbase["r"] + 3 * ch + co] = b_ih[sc][co] + b_hh[sc][co]
            wg[24, mbase["z"] + 3 * ch + co] = b_ih[sc][3 + co] + b_hh[sc][3 + co]
            wg[24, mbase["inn"] + 3 * ch + co] = b_ih[sc][6 + co]
            wg[24, mbase["hnh"] + 3 * ch + co] = b_hh[sc][6 + co]

    # conv taps, replicated at the 4 group bases (32g); K rows 32g+(0..23)
    wcv = np.zeros((128, 3, RW), np.float32)
    for gq in range(4):
        for ch in range(CH):
            sc = ch // B
            for t in range(3):
                for ci in range(C):
                    for co in range(C):
                        wcv[32 * gq + 3 * ch + ci, t, 3 * ch + co] = cv_w[sc][co, ci, t]
    cvb = np.zeros((RW, 1), np.float32)  # conv bias (via ACT tanh bias operand)
    for ch in range(CH):
        for co in range(C):
            cvb[3 * ch + co, 0] = cv_b[ch // B][co]

    # combine lhsT's: K = full 128 ctx partitions, M = 48 rows (g, b, o)
    wa = np.zeros((128, 48), np.float32)
    wl = np.zeros((128, 48), np.float32)
    for gq in range(4):
        for b in range(B):
            for o in range(C):
                m = 12 * gq + 3 * b + o
                for ci in range(C):
                    wa[32 * gq + 3 * b + ci, m] = cb_w[o, ci]  # row-scan rows
                    wl[32 * gq + 12 + 3 * b + ci, m] = cb_w[o, C + ci]  # col rows
    cbb = np.zeros((48, 1), np.float32)  # combine bias (via final sigmoid bias)
    for gq in range(4):
        for b in range(B):
            for o in range(C):
                cbb[12 * gq + 3 * b + o, 0] = cb_b[o]
    wpk = np.zeros((128, 304), np.float32)
    wpk[:KG, 0:128] = wg
    wpk[:, 128:200] = wcv.reshape(128, 72)
    wpk[:, 200:248] = wa
    wpk[:, 248:296] = wl
    wpk[32:56, 296] = cvb[:, 0]
    wpk[0:48, 297] = cbb[:, 0]
    return {"wpk": wpk.astype(BF16)}


def _pack_x2(x_core):
    """x_core: (B, C, S, S) f32 -> (25, S, S) bf16 matmul-ready rows.
    Row 3*chain+c: row-scan chains get x[b,c,step,pos]; col-scan chains
    get x[b,c,pos,step] (transposed plane). Row 24 = ones (bias source)."""
    S = x_core.shape[2]
    x2 = np.empty((25, S, S), np.float32)
    for b in range(B):
        for ci in range(C):
            x2[3 * b + ci] = x_core[b, ci]
            x2[12 + 3 * b + ci] = x_core[b, ci].T
    x2[24] = 1.0
    return x2.astype(BF16)


def build_nc(S, lt_pmajor=False):
    """Build the Bass module for per-core sizes (B=4, C=3, H=W=S).
    lt_pmajor: xbar transpose output h-index mapping h = p*J + j if True
    else h = j*P + p (determined empirically in sim)."""
    import concourse.bass as bass
    import concourse.bacc as bacc
    import concourse.mybir as mybir
    from concourse.tile import TileContext
    from contextlib import ExitStack

    fp32 = mybir.dt.float32
    bf16 = mybir.dt.bfloat16
    AF = mybir.ActivationFunctionType
    OP = mybir.AluOpType
    KSLOTS = S // 4  # ctx slots per group

    nc = bacc.Bacc()
    x2_d = nc.declare_dram_parameter("x2", [25, S, S], bf16, isOutput=False)
    wpk_d = nc.declare_dram_parameter("wpk", [128, 304], bf16, isOutput=False)
    out_d = nc.declare_dram_parameter("out", [B, C, S, S], fp32, isOutput=True)
    lp_d = nc.dram_tensor("lpart", [B, C, S, S], bf16)  # [b, o, w, h]
    lpt_d = nc.dram_tensor("lpartT", [B, C, S, S], bf16)  # [b, o, h, w]

    with TileContext(nc) as tc, ExitStack() as es:
        cst = es.enter_context(tc.tile_pool(name="cst", bufs=1))
        big = es.enter_context(tc.tile_pool(name="big", bufs=1))
        wrk = es.enter_context(tc.tile_pool(name="wrk", bufs=2))
        psg = es.enter_context(tc.tile_pool(name="psg", bufs=2, space="PSUM"))
        psc = es.enter_context(tc.tile_pool(name="psc", bufs=2, space="PSUM"))
        psx = es.enter_context(tc.tile_pool(name="psx", bufs=2, space="PSUM"))

        wpk = cst.tile([128, 304], bf16)
        nc.sync.dma_start(out=wpk[:], in_=wpk_d[:])
        wg = wpk[0:KG, 0:128]
        wcv = wpk[:, 128:200].rearrange("p (t m) -> p t m", t=3)
        wa = wpk[:, 200:248]
        wl = wpk[:, 248:296]
        cvb = wpk[32:56, 296:297]
        cbb = wpk[0:48, 297:298]

        # persistent state
        xh = big.tile([KG, NSLOT, S], bf16)  # x 0-23, ones 24, h 32-55
        ctx = big.tile([128, KSLOTS, S + 2], bf16)
        for k0 in range(0, KSLOTS, 64):
            nc.vector.memset(ctx[:, k0 : min(k0 + 64, KSLOTS), :], 0.0)
        nc.vector.memset(xh[:], 0.0)  # pads stay 0; also h0 = 0

        # ---------------- scan loop ----------------
        for s in range(S):
            sl = s % NSLOT
            gq = s % 4
            slot = s // 4
            if s % CHUNK == 0:
                nc.sync.dma_start(
                    out=xh[0:25, sl : sl + CHUNK, :],
                    in_=x2_d[:, s : s + CHUNK, :],
                )
            pg = psg.tile([MG, S], fp32)
            nc.tensor.matmul(pg[:], wg, xh[:, sl, :], start=True, stop=True)
            rz = wrk.tile([KG, S], bf16, tag="rz")  # r 0-23, z 32-55
            nc.scalar.activation(rz[:], pg[0:KG, :], AF.Sigmoid)
            t1 = wrk.tile([KG, S], fp32, tag="t1")
            nc.vector.tensor_tensor(t1[32:56, :], rz[0:RW, :], pg[96:120, :], OP.mult)
            t2 = wrk.tile([KG, S], fp32, tag="t2")
            nc.vector.tensor_tensor(t2[32:56, :], t1[32:56, :], pg[64:88, :], OP.add)
            # n, d, e live at partition base 32 so SBUF+SBUF tensor_tensor
            # operand pairs share a start partition (walrus checkSBSameStartPartition)
            n = wrk.tile([KG, S], bf16, tag="n")
            nc.scalar.activation(n[32:56, :], t2[32:56, :], AF.Tanh)
            d = wrk.tile([KG, S], bf16, tag="d")
            nc.vector.tensor_tensor(d[32:56, :], xh[32:56, sl, :], n[32:56, :], OP.subtract)
            e = wrk.tile([KG, S], bf16, tag="e")
            nc.vector.tensor_tensor(e[32:56, :], rz[32:56, :], d[32:56, :], OP.mult)
            # ctx_s = n + e  (the GRU output, stored context)
            nc.vector.tensor_tensor(
                ctx[32 * gq : 32 * gq + RW, slot, 1 : S + 1], n[32:56, :], e[32:56, :], OP.add
            )
            pc = psc.tile([KG, S], fp32)
            for t in range(3):
                nc.tensor.matmul(
                    pc[32:56, :],
                    wcv[32 * gq : 32 * gq + RW, t, :],
                    ctx[32 * gq : 32 * gq + RW, slot, t : t + S],
                    start=(t == 0),
                    stop=(t == 2),
                    tile_position=(32 * gq, 32),
                )
            nc.scalar.activation(
                xh[32:56, (s + 1) % NSLOT, :], pc[32:56, :], AF.Tanh, bias=cvb
            )

        # ---------------- pass 1: L-part (col-scan combine half) ----------------
        for k in range(KSLOTS):
            pl = psx.tile([48, S], fp32, tag="pl")
            nc.tensor.matmul(pl[:], wl, ctx[:, k, 1 : S + 1], start=True, stop=True)
            lsb = wrk.tile([48, S], bf16, tag="lsb")
            if k % 2 == 0:
                nc.vector.tensor_copy(lsb[:], pl[:])
            else:
                nc.scalar.copy(lsb[:], pl[:])
            # dest rows (g', b, o) -> lpart[b, o, w=4k+g', :]
            lp_v = lp_d.rearrange("b o (k g) h -> g b o k h", g=4)
            nc.sync.dma_start(out=lp_v[:, :, :, k, :], in_=lsb[:])

        # ---------------- xbar transpose: lpart[b,o] (S_w, S_h) -> h-major ----------------
        P = min(S, 128)
        J = S // P
        for b in range(B):
            for o in range(C):
                ltb = wrk.tile([P, J, S], bf16, tag="ltb")
                nc.sync.dma_start_transpose(ltb[:], lp_d[b, o, :, :])
                if lt_pmajor:  # h = p*J + j
                    lpt_v = lpt_d[b, o].rearrange("(p j) w -> p j w", j=J)
                else:  # h = j*P + p
                    lpt_v = lpt_d[b, o].rearrange("(j p) w -> p j w", p=P)
                nc.sync.dma_start(out=lpt_v, in_=ltb[:])

        # ---------------- pass 2: A-part + L + sigmoid -> out ----------------
        lpt_g = lpt_d.rearrange("b o (k g) w -> g b o k w", g=4)
        out_g = out_d.rearrange("b o (k g) w -> g b o k w", g=4)
        for k in range(KSLOTS):
            l2 = wrk.tile([48, S], bf16, tag="l2")
            nc.sync.dma_start(out=l2[:], in_=lpt_g[:, :, :, k, :])
            pa = psx.tile([48, S], fp32, tag="pa")
            nc.tensor.matmul(pa[:], wa, ctx[:, k, 1 : S + 1], start=True, stop=True)
            osum = wrk.tile([48, S], fp32, tag="osum")
            nc.vector.tensor_tensor(osum[:], pa[:], l2[:], OP.add)
            ot = wrk.tile([48, S], fp32, tag="ot")
            nc.scalar.activation(ot[:], osum[:], AF.Sigmoid, bias=cbb)
            nc.sync.dma_start(out=out_g[:, :, :, k, :], in_=ot[:])
    nc.compile()
    return nc


def _run(x, packed, S, trace=False, nc=None):
    """Shard over 8 cores, run, gather. x: (8B, C, S, S) f32."""
    from concourse.bass_utils import run_bass_kernel_spmd

    if nc is None:
        nc = build_nc(S)
    in_maps = []
    for core in range(NCORES):
        xc = x[core * B : (core + 1) * B]
        in_maps.append({"x2": _pack_x2(xc), **packed})
    core_ids = list(range(NCORES))
    res = run_bass_kernel_spmd(nc, in_maps, core_ids, trace=trace)
    outs = [res.results[i]["out"] for i in range(NCORES)]
    return np.concatenate(outs, axis=0), res


def kernel(**inputs):
    x = np.asarray(inputs["x"], np.float32)
    packed = _pack_weights(
        {k: np.asarray(v, np.float32) for k, v in inputs.items() if k != "x"}
    )
    out, _ = _run(x, packed, x.shape[2])
    return out.astype(np.float32)


# revision 24
# speedup vs baseline: 12.7170x; 12.7170x over previous
"""CSRN (spatial GRU scan network) Trainium2 kernel.

Sharding: data-parallel over batch across 8 cores (4 batches/core).
Each core runs both directional scans (row + col) fused: 8 chains
(2 scans x 4 batches) stepped together, 512 sequential steps.

Host-side prep (not counted in HW time): packs x into a bf16 tensor
holding both the natural (row-scan) and transposed (col-scan) layouts
as matmul-ready contraction rows plus a constant ones-row (for biases),
and packs the tiny GRU/conv/combine weights into block-structured lhsT
matrices.

Layout rules honored: engine SBUF access patterns start at partition
0/32/64/96 only (PSUM operands are exempt); matmul lhsT/rhs share a
32-aligned base partition; one matmul output <= one PSUM bank (N<=512).
"""

import sys

sys.path.insert(0, "/opt/trn_rl_repo")

import numpy as np
import ml_dtypes

BF16 = ml_dtypes.bfloat16

C = 3  # channels
B = 4  # batches per core
NCORES = 8
CH = 2 * B  # chains per core (2 scans x 4 batches)
RW = CH * C  # 24 data rows per class
KG = 56  # gate-matmul contraction rows: x 0-23, ones 24, (pad), h 32-55
MG = 128  # gate M rows: r 0-23, z 32-55, inn 64-87, hnh 96-119 (rest pad)
NSLOT = 32  # xh ring slots
CHUNK = 16  # x-stream DMA chunk (steps)


def _pack_weights(inp):
    """Build lhsT matrices (numpy fp32 -> bf16). Row order everywhere:
    chain = scan*4 + b (scan 0 = row scan, 1 = col scan), row = 3*chain + c."""
    w_ih = [inp["w_ih_rows"], inp["w_ih_cols"]]
    w_hh = [inp["w_hh_rows"], inp["w_hh_cols"]]
    b_ih = [inp["b_ih_rows"], inp["b_ih_cols"]]
    b_hh = [inp["b_hh_rows"], inp["b_hh_cols"]]
    cv_w = [inp["conv_rows_w"], inp["conv_cols_w"]]
    cv_b = [inp["conv_rows_b"], inp["conv_cols_b"]]
    cb_w = inp["combine_w"]  # (C, 2C)
    cb_b = inp["combine_b"]  # (C,)

    wg = np.zeros((KG, MG), np.float32)
    mbase = {"r": 0, "z": 32, "inn": 64, "hnh": 96}
    for ch in range(CH):
        sc = ch // B
        for ci in range(C):
            kx = 3 * ch + ci  # x row
            kh = 32 + 3 * ch + ci  # h row
            for co in range(C):
                wg[kx, mbase["r"] + 3 * ch + co] = w_ih[sc][0 + co, ci]
                wg[kh, mbase["r"] + 3 * ch + co] = w_hh[sc][0 + co, ci]
                wg[kx, mbase["z"] + 3 * ch + co] = w_ih[sc][3 + co, ci]
                wg[kh, mbase["z"] + 3 * ch + co] = w_hh[sc][3 + co, ci]
                wg[kx, mbase["inn"] + 3 * ch + co] = w_ih[sc][6 + co, ci]
                wg[kh, mbase["hnh"] + 3 * ch + co] = w_hh[sc][6 + co, ci]
        for co in range(C):  # biases via the ones row (k=24)
            wg[24, mbase["r"] + 3 * ch + co] = b_ih[sc][co] + b_hh[sc][co]
            wg[24, mbase["z"] + 3 * ch + co] = b_ih[sc][3 + co] + b_hh[sc][3 + co]
            wg[24, mbase["inn"] + 3 * ch + co] = b_ih[sc][6 + co]
            wg[24, mbase["hnh"] + 3 * ch + co] = b_hh[sc][6 + co]

    # conv taps, replicated at the 4 group bases (32g); K rows 32g+(0..23)
    wcv = np.zeros((128, 3, RW), np.float32)
    for gq in range(4):
        for ch in range(CH):
            sc = ch // B
            for t in range(3):
                for ci in range(C):
                    for co in range(C):
                        wcv[32 * gq + 3 * ch + ci, t, 3 * ch + co] = cv_w[sc][co, ci, t]
    cvb = np.zeros((RW, 1), np.float32)  # conv bias (via ACT tanh bias operand)
    for ch in range(CH):
        for co in range(C):
            cvb[3 * ch + co, 0] = cv_b[ch // B][co]

    # combine lhsT's: K = full 128 ctx partitions, M = 48 rows (g, b, o)
    wa = np.zeros((128, 48), np.float32)
    wl = np.zeros((128, 48), np.float32)
    for gq in range(4):
        for b in range(B):
            for o in range(C):
                m = 12 * gq + 3 * b + o
                for ci in range(C):
                    wa[32 * gq + 3 * b + ci, m] = cb_w[o, ci]  # row-scan rows
                    wl[32 * gq + 12 + 3 * b + ci, m] = cb_w[o, C + ci]  # col rows
    cbb = np.zeros((48, 1), np.float32)  # combine bias (via final sigmoid bias)
    for gq in range(4):
        for b in range(B):
            for o in range(C):
                cbb[12 * gq + 3 * b + o, 0] = cb_b[o]
    wpk = np.zeros((128, 304), np.float32)
    wpk[:KG, 0:128] = wg
    wpk[:, 128:200] = wcv.reshape(128, 72)
    wpk[:, 200:248] = wa
    wpk[:, 248:296] = wl
    wpk[32:56, 296] = cvb[:, 0]
    wpk[0:48, 297] = cbb[:, 0]
    return {"wpk": wpk.astype(BF16)}


def _pack_x2(x_core):
    """x_core: (B, C, S, S) f32 -> (25, S, S) bf16 matmul-ready rows.
    Row 3*chain+c: row-scan chains get x[b,c,step,pos]; col-scan chains
    get x[b,c,pos,step] (transposed plane). Row 24 = ones (bias source)."""
    S = x_core.shape[2]
    x2 = np.empty((25, S, S), np.float32)
    for b in range(B):
        for ci in range(C):
            x2[3 * b + ci] = x_core[b, ci]
            x2[12 + 3 * b + ci] = x_core[b, ci].T
    x2[24] = 1.0
    return x2.astype(BF16)


def build_nc(S, lt_pmajor=False):
    """Build the Bass module for per-core sizes (B=4, C=3, H=W=S).
    lt_pmajor: xbar transpose output h-index mapping h = p*J + j if True
    else h = j*P + p (determined empirically in sim)."""
    import concourse.bass as bass
    import concourse.bacc as bacc
    import concourse.mybir as mybir
    from concourse.tile import TileContext
    from contextlib import ExitStack

    fp32 = mybir.dt.float32
    bf16 = mybir.dt.bfloat16
    AF = mybir.ActivationFunctionType
    OP = mybir.AluOpType
    KSLOTS = S // 4  # ctx slots per group

    nc = bacc.Bacc()
    x2_d = nc.declare_dram_parameter("x2", [25, S, S], bf16, isOutput=False)
    wpk_d = nc.declare_dram_parameter("wpk", [128, 304], bf16, isOutput=False)
    out_d = nc.declare_dram_parameter("out", [B, C, S, S], fp32, isOutput=True)
    lp_d = nc.dram_tensor("lpart", [B, C, S, S], bf16)  # [b, o, w, h]
    lpt_d = nc.dram_tensor("lpartT", [B, C, S, S], bf16)  # [b, o, h, w]

    with TileContext(nc) as tc, ExitStack() as es:
        cst = es.enter_context(tc.tile_pool(name="cst", bufs=1))
        big = es.enter_context(tc.tile_pool(name="big", bufs=1))
        wrk = es.enter_context(tc.tile_pool(name="wrk", bufs=2))
        psg = es.enter_context(tc.tile_pool(name="psg", bufs=1, space="PSUM"))
        psc = es.enter_context(tc.tile_pool(name="psc", bufs=1, space="PSUM"))
        psx = es.enter_context(tc.tile_pool(name="psx", bufs=2, space="PSUM"))

        wpk = cst.tile([128, 304], bf16)
        nc.sync.dma_start(out=wpk[:], in_=wpk_d[:])
        wg = wpk[0:KG, 0:128]
        wcv = wpk[:, 128:200].rearrange("p (t m) -> p t m", t=3)
        wa = wpk[:, 200:248]
        wl = wpk[:, 248:296]
        cvb = wpk[32:56, 296:297]
        cbb = wpk[0:48, 297:298]

        # persistent state
        xh = big.tile([KG, NSLOT, S], bf16)  # x 0-23, ones 24, h 32-55
        ctx = big.tile([128, KSLOTS, S + 2], bf16)
        for k0 in range(0, KSLOTS, 64):
            nc.vector.memset(ctx[:, k0 : min(k0 + 64, KSLOTS), :], 0.0)
        nc.vector.memset(xh[:], 0.0)  # pads stay 0; also h0 = 0

        # ---------------- scan loop ----------------
        for s in range(S):
            sl = s % NSLOT
            gq = s % 4
            slot = s // 4
            if s % CHUNK == 0:
                nc.sync.dma_start(
                    out=xh[0:25, sl : sl + CHUNK, :],
                    in_=x2_d[:, s : s + CHUNK, :],
                )
            # position-half split: two half-chains (F=S/2) pipeline across engines
            H2 = S // 2
            pg0 = psg.tile([MG, H2], fp32, tag="pg0", name="pg0")
            pg1 = psg.tile([MG, H2], fp32, tag="pg1", name="pg1")
            pc0 = psc.tile([KG, H2], fp32, tag="pc0", name="pc0")
            pc1 = psc.tile([KG, H2], fp32, tag="pc1", name="pc1")
            pgs = [pg0, pg1]
            pcs = [pc0, pc1]
            rz = wrk.tile([KG, S], bf16, tag="rz")  # r 0-23, z 32-55
            t1 = wrk.tile([KG, S], fp32, tag="t1")
            t2 = wrk.tile([KG, S], fp32, tag="t2")
            n = wrk.tile([KG, S], bf16, tag="n")
            d = wrk.tile([KG, S], bf16, tag="d")
            e = wrk.tile([KG, S], bf16, tag="e")
            for hf in range(2):
                lo, hi = hf * H2, (hf + 1) * H2
                pg = pgs[hf]
                nc.tensor.matmul(pg[:], wg, xh[:, sl, lo:hi], start=True, stop=True)
                nc.scalar.activation(rz[:, lo:hi], pg[0:KG, :], AF.Sigmoid)
                nc.vector.tensor_tensor(t1[32:56, lo:hi], rz[0:RW, lo:hi], pg[96:120, :], OP.mult)
                nc.vector.tensor_tensor(pg[64:88, :], t1[32:56, lo:hi], pg[64:88, :], OP.add)
                nc.scalar.activation(n[32:56, lo:hi], pg[64:88, :], AF.Tanh)
                nc.vector.tensor_tensor(d[32:56, lo:hi], xh[32:56, sl, lo:hi], n[32:56, lo:hi], OP.subtract)
                nc.vector.tensor_tensor(e[32:56, lo:hi], rz[32:56, lo:hi], d[32:56, lo:hi], OP.mult)
                # ctx_s = n + z*(h-n)  (the GRU output, stored context)
                nc.vector.tensor_tensor(
                    ctx[32 * gq : 32 * gq + RW, slot, 1 + lo : 1 + hi],
                    n[32:56, lo:hi], e[32:56, lo:hi], OP.add
                )
            for hf in range(2):
                lo, hi = hf * H2, (hf + 1) * H2
                pc = pcs[hf]
                for t in range(3):
                    nc.tensor.matmul(
                        pc[32:56, :],
                        wcv[32 * gq : 32 * gq + RW, t, :],
                        ctx[32 * gq : 32 * gq + RW, slot, t + lo : t + lo + H2],
                        start=(t == 0),
                        stop=(t == 2),
                        tile_position=(32 * gq, 32),
                    )
                nc.scalar.activation(
                    xh[32:56, (s + 1) % NSLOT, lo:hi], pc[32:56, :], AF.Tanh, bias=cvb
                )
            # ---- pass 1 (L-part) interleaved at low priority (gap filler) ----
            if gq == 3:
                k = slot
                _prio = tc.cur_priority
                tc.cur_priority = 10_000_000 + k
                pl = psx.tile([48, S], fp32, tag="pl")
                nc.tensor.matmul(pl[:], wl, ctx[:, k, 1 : S + 1], start=True, stop=True)
                lsb = wrk.tile([48, S], bf16, tag="lsb")
                if k % 2 == 0:
                    nc.vector.tensor_copy(lsb[:], pl[:])
                else:
                    nc.scalar.copy(lsb[:], pl[:])
                # dest rows (g', b, o) -> lpart[b, o, w=4k+g', :]
                lp_v = lp_d.rearrange("b o (k g) h -> g b o k h", g=4)
                nc.sync.dma_start(out=lp_v[:, :, :, k, :], in_=lsb[:])
                tc.cur_priority = _prio

        # ---------------- xbar transpose: lpart[b,o] (S_w, S_h) -> h-major ----------------
        P = min(S, 128)
        J = S // P
        for b in range(B):
            for o in range(C):
                ltb = wrk.tile([P, J, S], bf16, tag="ltb")
                nc.sync.dma_start_transpose(ltb[:], lp_d[b, o, :, :])
                if lt_pmajor:  # h = p*J + j
                    lpt_v = lpt_d[b, o].rearrange("(p j) w -> p j w", j=J)
                else:  # h = j*P + p
                    lpt_v = lpt_d[b, o].rearrange("(j p) w -> p j w", p=P)
                nc.sync.dma_start(out=lpt_v, in_=ltb[:])

        # ---------------- pass 2: A-part + L + sigmoid -> out ----------------
        lpt_g = lpt_d.rearrange("b o (k g) w -> g b o k w", g=4)
        out_g = out_d.rearrange("b o (k g) w -> g b o k w", g=4)
        for k in range(KSLOTS):
            l2 = wrk.tile([48, S], bf16, tag="l2")
            nc.sync.dma_start(out=l2[:], in_=lpt_g[:, :, :, k, :])
            pa = psx.tile([48, S], fp32, tag="pa")
            nc.tensor.matmul(pa[:], wa, ctx[:, k, 1 : S + 1], start=True, stop=True)
            osum = wrk.tile([48, S], fp32, tag="osum")
            nc.vector.tensor_tensor(osum[:], pa[:], l2[:], OP.add)
            ot = wrk.tile([48, S], fp32, tag="ot")
            nc.scalar.activation(ot[:], osum[:], AF.Sigmoid, bias=cbb)
            nc.sync.dma_start(out=out_g[:, :, :, k, :], in_=ot[:])
    nc.compile()
    return nc


def _run(x, packed, S, trace=False, nc=None):
    """Shard over 8 cores, run, gather. x: (8B, C, S, S) f32."""
    from concourse.bass_utils import run_bass_kernel_spmd

    if nc is None:
        nc = build_nc(S)
    in_maps = []
    for core in range(NCORES):
        xc = x[core * B : (core + 1) * B]
        in_maps.append({"x2": _pack_x2(xc), **packed})
    core_ids = list(range(NCORES))
    res = run_bass_kernel_spmd(nc, in_maps, core_ids, trace=trace)
    outs = [res.results[i]["out"] for i in range(NCORES)]
    return np.concatenate(outs, axis=0), res


def kernel(**inputs):
    x = np.asarray(inputs["x"], np.float32)
    packed = _pack_weights(
        {k: np.asarray(v, np.float32) for k, v in inputs.items() if k != "x"}
    )
    out, _ = _run(x, packed, x.shape[2])
    return out.astype(np.float32)
